# revision 1
# baseline (speedup 1.0000x reference)
"""Trainium2 Bass kernel for a full causal MHA layer (B=2, T=2048, C=2048, H=16,
partial RoPE on first 64 dims of each 128-dim head).

Sharding over 8 cores: core c handles batch b=c//4 and heads [4g, 4g+4), g=c%4
(tensor-parallel over heads x data-parallel over batch). Each core:
  phase 1: q/k/v projections (fp32r matmuls), bias, partial RoPE on q/k,
           spill qT/kT [d,t] and v [t,m] to DRAM scratch
  phase 2: per (i-chunk, head), causal attention in transposed layout:
           scoresT[j,i] -> exp (no max subtraction; causal logits peak ~9.5)
           -> causal mask on diagonal tiles -> out accumulation outT[d,i]
           with row-sums via a ones-matmul -> normalize -> attnT[m,t]
  phase 3: output projection partial outT[c,t], per t-chunk (overlaps ph2 tail)
Host: slices inputs per core, sums the 4 TP partials per batch, adds bo.
"""

import math

import numpy as np

import concourse.bass as bass
import concourse.mybir as mybir
import concourse.tile as tile
from concourse import bacc
from concourse.bass_utils import run_bass_kernel_spmd

F32 = mybir.dt.float32
F32R = mybir.dt.float32r

B, T, C = 2, 2048, 2048
H = 16
HS = 128
ROT = 64
HALF = 32
BASE = 10000.0

N_CORES = 8
TPG = 4                # TP group size (heads split)
H_LOC = H // TPG       # 4 heads per core
M = H_LOC * HS         # 512 local head-dim columns
SCALE = 1.0 / math.sqrt(HS)

P = 128
NT = T // 512          # 4 t-chunks of 512
CT = C // P            # 16 contraction tiles
JT = T // P            # 16 key tiles per head

_NC_CACHE = {}


def _build(phases=(1, 2, 3)):
    nc = bacc.Bacc(None, target_bir_lowering=False)

    xT = nc.declare_dram_parameter("xT", [C, T], F32R, isOutput=False)
    wqT = nc.declare_dram_parameter("wqT", [C, M], F32R, isOutput=False)
    wkT = nc.declare_dram_parameter("wkT", [C, M], F32R, isOutput=False)
    wvT = nc.declare_dram_parameter("wvT", [C, M], F32R, isOutput=False)
    woT = nc.declare_dram_parameter("woT", [M, C], F32R, isOutput=False)
    bqc = nc.declare_dram_parameter("bqc", [P, H_LOC], F32, isOutput=False)
    bkc = nc.declare_dram_parameter("bkc", [P, H_LOC], F32, isOutput=False)
    bvr = nc.declare_dram_parameter("bvr", [1, M], F32R, isOutput=False)
    cosT = nc.declare_dram_parameter("cosT", [ROT, T], F32, isOutput=False)
    ones1_d = nc.declare_dram_parameter("ones1", [1, P], F32R, isOutput=False)
    ones128_d = nc.declare_dram_parameter("ones128", [P, 1], F32R, isOutput=False)
    nsT = nc.declare_dram_parameter("nsT", [ROT, T], F32, isOutput=False)
    outT = nc.declare_dram_parameter("outT", [C, T], F32, isOutput=True)

    qt_d = nc.dram_tensor("qt_scratch", [H_LOC, P, T], F32R)
    v_d = nc.dram_tensor("v_scratch", [T, M], F32R)

    with tile.TileContext(nc) as tc, \
         tc.tile_pool(name="const", bufs=1) as const:
        cos_sb = const.tile([ROT, T], F32, tag="cos")
        ns_sb = const.tile([ROT, T], F32, tag="ns")
        bq_sb = const.tile([P, H_LOC], F32, tag="bq")
        bk_sb = const.tile([P, H_LOC], F32, tag="bk")
        bv_sb = const.tile([1, M], F32R, tag="bv")
        ones1 = const.tile([1, P], F32R, tag="ones1")
        ones128 = const.tile([P, 1], F32R, tag="ones128")
        bvb_sb = const.tile([P, M], F32R, tag="bvb")
        k_res = const.tile([P, H_LOC, T], F32R, tag="kres")
        nc.sync.dma_start(out=cos_sb[:], in_=cosT[:])
        nc.sync.dma_start(out=ns_sb[:], in_=nsT[:])
        nc.sync.dma_start(out=bq_sb[:], in_=bqc[:])
        nc.sync.dma_start(out=bk_sb[:], in_=bkc[:])
        nc.sync.dma_start(out=bv_sb[:], in_=bvr[:])
        nc.sync.dma_start(out=ones1[:], in_=ones1_d[:])
        nc.sync.dma_start(out=ones128[:], in_=ones128_d[:])

        # ---------------- phase 1: projections ----------------
        if 1 in phases:
         with tc.tile_pool(name="p1w", bufs=CT) as wpool, \
             tc.tile_pool(name="p1x", bufs=16) as xpool, \
             tc.tile_pool(name="p1e", bufs=2) as epool, \
             tc.tile_pool(name="p1r", bufs=2) as rpool, \
             tc.tile_pool(name="p1psqk", bufs=6, space="PSUM") as psqkpool, \
             tc.tile_pool(name="p1psv", bufs=2, space="PSUM") as psvpool:
            # broadcast bv across 128 partitions once via stride-0 DMA read
            nc.sync.dma_start(out=bvb_sb[:], in_=bvr[0:1, :].to_broadcast([P, M]))

            wq_t = [wpool.tile([P, M], F32R, tag="wq", name=f"wq{i}") for i in range(CT)]
            wk_t = [wpool.tile([P, M], F32R, tag="wk", name=f"wk{i}") for i in range(CT)]
            wv_t = [wpool.tile([P, M], F32R, tag="wv", name=f"wv{i}") for i in range(CT)]
            # first x chunk before weights so the first matmul group starts early
            x0_t = [xpool.tile([P, 512], F32R, tag="x", name=f"x0_{i}")
                    for i in range(CT)]
            for ct in range(CT):
                nc.sync.dma_start(out=x0_t[ct][:], in_=xT[ct * P:(ct + 1) * P, 0:512])
                nc.sync.dma_start(out=wq_t[ct][:], in_=wqT[ct * P:(ct + 1) * P, :])
            x1_t = [xpool.tile([P, 512], F32R, tag="x", name=f"x1_{i}")
                    for i in range(CT)]
            for ct in range(CT):
                nc.sync.dma_start(out=wk_t[ct][:], in_=wkT[ct * P:(ct + 1) * P, :])
                nc.sync.dma_start(out=x1_t[ct][:],
                                  in_=xT[ct * P:(ct + 1) * P, 512:1024])
                nc.sync.dma_start(out=wv_t[ct][:], in_=wvT[ct * P:(ct + 1) * P, :])

            for tch in range(NT):
                ts0 = tch * 512
                if tch == 0:
                    x_t = x0_t
                elif tch == 1:
                    x_t = x1_t
                else:
                    x_t = [xpool.tile([P, 512], F32R, tag="x", name=f"x{tch}_{i}")
                           for i in range(CT)]
                    for ct in range(CT):
                        nc.sync.dma_start(
                            out=x_t[ct][:],
                            in_=xT[ct * P:(ct + 1) * P, ts0:ts0 + 512])

                for proj, w_t, b_sb in (("q", wq_t, bq_sb), ("k", wk_t, bk_sb)):
                    for mt in range(H_LOC):
                        ps = psqkpool.tile([P, 512], F32, tag="psqk")
                        for ct in range(CT):
                            nc.tensor.matmul(
                                ps[:],
                                lhsT=w_t[ct][:, mt * P:(mt + 1) * P],
                                rhs=x_t[ct][:],
                                start=(ct == 0), stop=(ct == CT - 1))
                        if proj == "q":
                            # full biased evict, rope rows 0..63, spill to DRAM
                            qtmp = epool.tile([P, 512], F32R, tag="qtmp")
                            nc.scalar.activation(
                                qtmp[:], ps[:],
                                mybir.ActivationFunctionType.Identity,
                                bias=b_sb[:, mt:mt + 1], scale=1.0)
                            qsh = rpool.tile([ROT, 512], F32R, tag="qsh")
                            nc.sync.dma_start(out=qsh[0:HALF], in_=qtmp[HALF:ROT])
                            nc.sync.dma_start(out=qsh[HALF:ROT], in_=qtmp[0:HALF])
                            qrot = rpool.tile([ROT, 512], F32, tag="qrot")
                            nc.vector.tensor_tensor(
                                qrot[:], qsh[:], ns_sb[:, ts0:ts0 + 512],
                                mybir.AluOpType.mult)
                            tcos = rpool.tile([ROT, 512], F32, tag="tcos")
                            nc.vector.tensor_tensor(
                                tcos[:], qtmp[0:ROT], cos_sb[:, ts0:ts0 + 512],
                                mybir.AluOpType.mult)
                            nc.vector.tensor_tensor(
                                qtmp[0:ROT], tcos[:], qrot[:],
                                mybir.AluOpType.add)
                            nc.sync.dma_start(
                                out=qt_d[mt, :, ts0:ts0 + 512], in_=qtmp[:])
                        else:
                            # k stays in SBUF: rows 64..127 straight into k_res,
                            # rows 0..63 biased to tmp, rope, write into k_res
                            nc.scalar.activation(
                                k_res[ROT:P, mt, ts0:ts0 + 512], ps[ROT:P],
                                mybir.ActivationFunctionType.Identity,
                                bias=b_sb[ROT:P, mt:mt + 1], scale=1.0)
                            ktmp = epool.tile([ROT, 512], F32R, tag="ktmp")
                            nc.scalar.activation(
                                ktmp[:], ps[0:ROT],
                                mybir.ActivationFunctionType.Identity,
                                bias=b_sb[0:ROT, mt:mt + 1], scale=1.0)
                            ksh = rpool.tile([ROT, 512], F32R, tag="qsh")
                            nc.sync.dma_start(out=ksh[0:HALF], in_=ktmp[HALF:ROT])
                            nc.sync.dma_start(out=ksh[HALF:ROT], in_=ktmp[0:HALF])
                            krot = rpool.tile([ROT, 512], F32, tag="qrot")
                            nc.vector.tensor_tensor(
                                krot[:], ksh[:], ns_sb[:, ts0:ts0 + 512],
                                mybir.AluOpType.mult)
                            kcos = rpool.tile([ROT, 512], F32, tag="tcos")
                            nc.vector.tensor_tensor(
                                kcos[:], ktmp[:], cos_sb[:, ts0:ts0 + 512],
                                mybir.AluOpType.mult)
                            nc.vector.tensor_tensor(
                                k_res[0:ROT, mt, ts0:ts0 + 512], kcos[:], krot[:],
                                mybir.AluOpType.add)

                # v: [t_tile, m] layout, bias via broadcast add
                for tt in range(4):
                    ps = psvpool.tile([P, M], F32, tag="psv")
                    for ct in range(CT):
                        nc.tensor.matmul(
                            ps[:],
                            lhsT=x_t[ct][:, tt * P:(tt + 1) * P],
                            rhs=wv_t[ct][:],
                            start=(ct == 0), stop=(ct == CT - 1))
                    vtmp = epool.tile([P, M], F32R, tag="vtmp")
                    nc.vector.tensor_tensor(
                        vtmp[:], ps[:], bvb_sb[:], mybir.AluOpType.add)
                    t0 = ts0 + tt * P
                    nc.sync.dma_start(out=v_d[t0:t0 + P, :], in_=vtmp[:])

        # ---------------- phases 2+3 ----------------
        if 2 in phases:
         with tc.tile_pool(name="attn", bufs=1) as apool, \
             tc.tile_pool(name="p2v", bufs=1) as vpool, \
             tc.tile_pool(name="p2q", bufs=6) as qpool, \
             tc.tile_pool(name="p2e", bufs=6) as expool, \
             tc.tile_pool(name="p2d", bufs=2) as denpool, \
             tc.tile_pool(name="p3e", bufs=4) as oepool, \
             tc.tile_pool(name="p3w", bufs=H_LOC) as wopool, \
             tc.tile_pool(name="p2ps", bufs=3, space="PSUM") as ps2, \
             tc.tile_pool(name="p2psd", bufs=2, space="PSUM") as ps2d, \
             tc.tile_pool(name="p2psb", bufs=1, space="PSUM") as ps2b, \
             tc.tile_pool(name="p2pso", bufs=2, space="PSUM") as ps2o:
            attn_c = [apool.tile([P, H_LOC, 512], F32R, tag=f"attnT{i}",
                                 name=f"attn{i}") for i in range(NT)]
            v_r = v_d[:].rearrange("(jt p) m -> p jt m", p=P)
            vh_c = [[vpool.tile([P, 4, HS], F32R, tag=f"vh{h}_{jc}",
                                name=f"vh{h}_{jc}") for jc in range(NT)]
                    for h in range(H_LOC)]
            def load_vh(jc):
                for h in range(H_LOC):
                    nc.sync.dma_start(
                        out=vh_c[h][jc][:],
                        in_=v_r[:, 4 * jc:4 * jc + 4, h * HS:(h + 1) * HS])

            load_vh(0)

            wo_t = [wopool.tile([P, C], F32R, tag="wo", name=f"wo{i}")
                    for i in range(H_LOC)]

            for ic in range(NT):
                i0 = ic * 512
                njt = 4 * ic + 4
                qc_t = []
                for h in range(H_LOC):
                    qc = qpool.tile([P, 512], F32R, tag="qc", name=f"qc{ic}_{h}")
                    nc.sync.dma_start(out=qc[:], in_=qt_d[h, :, i0:i0 + 512])
                    qc_t.append(qc)
                if ic + 1 < NT:
                    load_vh(ic + 1)
                for h in range(H_LOC):
                    qc = qc_t[h]
                    ps_out = ps2o.tile([P, 512], F32, tag="psout")
                    ps_d = ps2d.tile([1, 512], F32, tag="psd")
                    for jt in range(njt):
                        ps_s = ps2.tile([P, 512], F32, tag="pss")
                        nc.tensor.matmul(
                            ps_s[:],
                            lhsT=k_res[:, h, jt * P:(jt + 1) * P],
                            rhs=qc[:],
                            start=True, stop=True)
                        ex = expool.tile([P, 512], F32R, tag="ex")
                        nc.scalar.activation(
                            ex[:], ps_s[:],
                            mybir.ActivationFunctionType.Exp, scale=SCALE)
                        if jt >= 4 * ic:
                            # keep where (i0 + il) - (jt*P + p) >= 0
                            nc.gpsimd.affine_select(
                                out=ex[:], in_=ex[:],
                                compare_op=mybir.AluOpType.is_ge,
                                fill=0.0,
                                base=i0 - jt * P,
                                channel_multiplier=-1,
                                pattern=[[1, 512]])
                        nc.tensor.matmul(
                            ps_out[:],
                            lhsT=vh_c[h][jt // 4][:, jt % 4, :],
                            rhs=ex[:],
                            start=(jt == 0), stop=(jt == njt - 1))
                        nc.tensor.matmul(
                            ps_d[:], lhsT=ones128[:], rhs=ex[:],
                            start=(jt == 0), stop=(jt == njt - 1))
                    # reciprocal straight from PSUM, rounded to fp32r
                    rrow = denpool.tile([1, 512], F32R, tag="rrow")
                    with nc.allow_low_precision(reason="softmax 1/den in fp32r"):
                        nc.vector.reciprocal(rrow[:], ps_d[:])
                    ps_b = ps2b.tile([P, 512], F32, tag="psb")
                    nc.tensor.matmul(ps_b[:], lhsT=ones1[:],
                                     rhs=rrow[:], start=True, stop=True)
                    rden = denpool.tile([P, 512], F32, tag="rden")
                    nc.vector.tensor_copy(out=rden[:], in_=ps_b[:])
                    nc.vector.tensor_tensor(
                        attn_c[ic][:, h, :], ps_out[:], rden[:],
                        mybir.AluOpType.mult)

                if ic == 0:
                    # deferred so boundary DMA bandwidth goes to vh/qc first
                    for mt in range(H_LOC):
                        nc.sync.dma_start(out=wo_t[mt][:],
                                          in_=woT[mt * P:(mt + 1) * P, :])
                # ---------------- phase 3 for this t-chunk ----------------
                if 3 in phases:
                    for co in range(CT):
                        ps = ps2.tile([P, 512], F32, tag="pss")
                        for mt in range(H_LOC):
                            nc.tensor.matmul(
                                ps[:],
                                lhsT=wo_t[mt][:, co * P:(co + 1) * P],
                                rhs=attn_c[ic][:, mt, :],
                                start=(mt == 0), stop=(mt == H_LOC - 1))
                        ot = oepool.tile([P, 512], F32, tag="ot")
                        nc.vector.tensor_copy(out=ot[:], in_=ps[:])
                        nc.sync.dma_start(
                            out=outT[co * P:(co + 1) * P, i0:i0 + 512],
                            in_=ot[:])

    nc.finalize()
    return nc


def get_nc(phases=(1, 2, 3)):
    if phases not in _NC_CACHE:
        _NC_CACHE[phases] = _build(phases)
    return _NC_CACHE[phases]


def _rope_tables():
    inv_freq = 1.0 / (BASE ** (np.arange(0, ROT, 2, dtype=np.float64) / ROT))
    freqs = np.arange(T, dtype=np.float64)[:, None] * inv_freq[None, :]  # [T, 32]
    cos_h = np.cos(freqs).T.astype(np.float32)   # [32, T]
    sin_h = np.sin(freqs).T.astype(np.float32)
    cosT = np.concatenate([cos_h, cos_h], axis=0)          # [64, T]
    nsT = np.concatenate([-sin_h, sin_h], axis=0)          # [64, T] signed sin
    return np.ascontiguousarray(cosT), np.ascontiguousarray(nsT)


def make_in_maps(x, Wq, bq, Wk, bk, Wv, bv, Wo, bo):
    cosT, nsT = _rope_tables()
    in_maps = []
    for c in range(N_CORES):
        b, g = divmod(c, TPG)
        ms = slice(g * M, (g + 1) * M)
        in_maps.append({
            "xT": np.ascontiguousarray(x[b].T),
            "wqT": np.ascontiguousarray(Wq[ms].T),
            "wkT": np.ascontiguousarray(Wk[ms].T),
            "wvT": np.ascontiguousarray(Wv[ms].T),
            "woT": np.ascontiguousarray(Wo[:, ms].T),
            "bqc": np.ascontiguousarray(bq[ms].reshape(H_LOC, P).T),
            "bkc": np.ascontiguousarray(bk[ms].reshape(H_LOC, P).T),
            "bvr": np.ascontiguousarray(bv[ms].reshape(1, M)),
            "cosT": cosT,
            "ones1": np.ones((1, P), np.float32),
            "ones128": np.ones((P, 1), np.float32),
            "nsT": nsT,
        })
    return in_maps


def assemble(results, bo):
    out = np.empty((B, T, C), dtype=np.float32)
    for b in range(B):
        acc = results[b * TPG]["outT"].astype(np.float32).copy()
        for g in range(1, TPG):
            acc += results[b * TPG + g]["outT"]
        out[b] = acc.T + bo[None, :]
    return out


def kernel(x, Wq, bq, Wk, bk, Wv, bv, Wo, bo):
    nc = get_nc()
    in_maps = make_in_maps(np.asarray(x, np.float32),
                           np.asarray(Wq, np.float32), np.asarray(bq, np.float32),
                           np.asarray(Wk, np.float32), np.asarray(bk, np.float32),
                           np.asarray(Wv, np.float32), np.asarray(bv, np.float32),
                           np.asarray(Wo, np.float32), np.asarray(bo, np.float32))
    res = run_bass_kernel_spmd(nc, in_maps, list(range(N_CORES)))
    return assemble(res.results, np.asarray(bo, np.float32))

